# revision 50
# baseline (speedup 1.0000x reference)
"""Trainium2 Bass kernel for nn_Attention_77025943487081.

Sharding: batch (4) data-parallel x 2-way head tensor-parallel over 8 cores.
Core c handles batch c//2 and heads [8*(c%2), 8*(c%2)+8). Each core produces a
partial c_proj output (contribution of its 512 input channels); the host adds
the two partials per batch plus the c_proj bias.

The per-head Conv2D projections, cross-head mixes, position projections and
biases are algebraically folded (on host) into dense bf16 matrices so the
device only runs plain matmuls. The 1/sqrt(D) score scale is folded into the
Q-side matrices.

Device schedule (flash-style, one pass over 512-seq blocks):
  for ss in 0..3:
    project q,k,v for seq block ss (bf16, accumulated over 9 contraction
    chunks of 128); q/k land transposed [dim, seq] in SBUF, v lands
    [seq, dim] with a ones-column appended (softmax denominator rides the
    PV matmul for free as output row 64)
    for each head: causal scores^T [k,q] (bf16 matmul) -> exp (ACT, chunks
    packed into 1024-wide PSUM tiles) -> causal mask of diagonal blocks on
    GpSimd -> PV accumulate po[d|1, q] -> reciprocal of row 64 (DVE),
    partition-broadcast (GpSimd), multiply into oT [hd, q] (DVE)
    partial c_proj of the previous block and the projections of the next
    block are interleaved between attention groups as PE filler so the PE
    never waits on exp/mask latency.
"""

import numpy as np
import ml_dtypes
from contextlib import ExitStack

import concourse.bass as bass
import concourse.tile as tile
from concourse import bacc, mybir
from concourse.bass_utils import run_bass_kernel_spmd

F32 = mybir.dt.float32
BF16 = mybir.dt.bfloat16
FP8 = mybir.dt.float8e4
EXP_SHIFT = -5.0  # exp(s-5): keeps fp8e4m3 probabilities in range

B, S, E, H, D, P = 4, 2048, 1024, 16, 64, 64
G = 8            # heads per core
NC = 8           # cores
EC = 9           # contraction chunks: 8 x 128 hidden + 1 (pos+bias, padded)
QKD = G * D      # 512 = per-core q (or k) width
NT = S // 128    # 16 seq tiles
ACT_EXP = mybir.ActivationFunctionType.Exp


def build_nc():
    nc = bacc.Bacc("TRN2", target_bir_lowering=False, debug=False, num_devices=NC)
    xT = nc.dram_tensor("xT", [EC, 128, S], BF16, kind="ExternalInput").ap()
    mqk = nc.dram_tensor("Mqk", [EC, 128, 2 * QKD], BF16, kind="ExternalInput").ap()
    mv = nc.dram_tensor("Mv", [EC, 128, QKD], BF16, kind="ExternalInput").ap()
    wc = nc.dram_tensor("Wc", [4, 128, E], BF16, kind="ExternalInput").ap()
    out = nc.dram_tensor("out", [S, E], F32, kind="ExternalOutput").ap()

    with nc.allow_low_precision("bf16 attention datapath"), \
         tile.TileContext(nc) as tc, ExitStack() as top:
        w_p = top.enter_context(tc.tile_pool(name="weights", bufs=1))
        xt_p = top.enter_context(tc.tile_pool(name="xt", bufs=2))
        qk_p = top.enter_context(tc.tile_pool(name="qkt", bufs=1))
        va_p = top.enter_context(tc.tile_pool(name="vaug", bufs=1))
        oT_p = top.enter_context(tc.tile_pool(name="oTt", bufs=4))
        pt_p = top.enter_context(tc.tile_pool(name="ptile", bufs=4))
        dg_p = top.enter_context(tc.tile_pool(name="diag", bufs=10))
        rc_p = top.enter_context(tc.tile_pool(name="rcp", bufs=4))
        bc_p = top.enter_context(tc.tile_pool(name="bcst", bufs=4))
        ost_p = top.enter_context(tc.tile_pool(name="ost", bufs=3))

        mqk_sb = w_p.tile([128, EC, 2 * QKD], BF16)
        mv_sb = w_p.tile([128, EC, QKD], BF16)
        wc_sb = w_p.tile([128, 4, E], BF16)
        qkt = [qk_p.tile([128, S], BF16, name=f"qkt{m}") for m in range(8)]
        v_aug = va_p.tile([128, NT, G, D + 1], BF16)
        # fp8 copy of v in (k-tile-pair, parity) layout for DoubleRow PV
        v8 = va_p.tile([128, NT // 2, G, 2, 80], FP8)

        eshift = w_p.tile([128, 1], F32)
        nc.vector.memset(v_aug[:, :, :, D:D + 1], 1.0)
        nc.vector.memset(v8[:, :, :, :, D:D + 1], 1.0)
        nc.vector.memset(eshift[:, :], EXP_SHIFT)

        # startup DMA, interleaved so the first contraction chunk lands first
        xts = [None] * 4
        xts[0] = xt_p.tile([128, EC, 512], BF16, tag="xt", name="xt0")
        for ec in range(EC):
            nc.sync.dma_start(out=mqk_sb[:, ec, :], in_=mqk[ec])
            nc.sync.dma_start(out=xts[0][:, ec, :], in_=xT[ec][:, 0:512])
        for ec in range(EC):
            nc.sync.dma_start(out=mv_sb[:, ec, :], in_=mv[ec])
        for gc in range(4):
            nc.sync.dma_start(out=wc_sb[:, gc, :], in_=wc[gc])

        def qk_mm(ps, m, xt, ec):
            nc.tensor.matmul(
                ps[:, :], mqk_sb[:, ec, m * 128:(m + 1) * 128], xt[:, ec, :],
                start=(ec == 0), stop=(ec == EC - 1))

        def v_mm(ps, sti, xt, ec):
            nc.tensor.matmul(
                ps[:, :], xt[:, ec, sti * 128:(sti + 1) * 128], mv_sb[:, ec, :],
                start=(ec == 0), stop=(ec == EC - 1))

        # persistent PSUM pools: 2 + 4 + 2 = 8 banks
        pp = top.enter_context(tc.tile_pool(name="pp", bufs=2, space="PSUM"))
        stp_p = top.enter_context(tc.tile_pool(name="stp", bufs=2, space="PSUM"))
        po_p = top.enter_context(tc.tile_pool(name="po", bufs=2, space="PSUM"))

        oTs = [None] * 4
        osts = {}

        def ph1_qk_wave(ss, w):
            ma, mb = w, 4 + w
            pa = pp.tile([128, 512], F32, tag="pp", name=f"qk{ss}w{w}a")
            pb = pp.tile([128, 512], F32, tag="pp", name=f"qk{ss}w{w}b")
            for ec in range(EC):
                qk_mm(pa, ma, xts[ss], ec)
                qk_mm(pb, mb, xts[ss], ec)
                if ec != EC - 1:
                    yield
            sl = slice(ss * 512, ss * 512 + 512)
            nc.vector.tensor_copy(qkt[ma][:, sl], pa[:, :])
            nc.vector.tensor_copy(qkt[mb][:, sl], pb[:, :])
            yield

        def ph1_v_wave(ss, w):
            for sti in (2 * w, 2 * w + 1):
                pv = pp.tile([128, 512], F32, tag="pp", name=f"v{ss}s{sti}")
                for ec in range(EC):
                    v_mm(pv, sti, xts[ss], ec)
                    if ec % 3 == 2 and ec != EC - 1:
                        yield
                stt = 4 * ss + sti
                nc.vector.tensor_copy(
                    v_aug[:, stt, :, 0:D],
                    pv[:, :].rearrange("p (g d) -> p g d", g=G))
                nc.vector.tensor_copy(
                    v8[:, stt // 2, :, stt % 2, 0:D],
                    pv[:, :].rearrange("p (g d) -> p g d", g=G))
                yield

        def ph3_tile(ss, qb, pool=None, ptag="pp"):
            """Partial c_proj for seq tile 4*ss+qb; yields every 2 matmuls."""
            if pool is None:
                pool = pp
            oT = oTs[ss]
            pca = pool.tile([128, 512], F32, tag=ptag, name=f"pc{ss}q{qb}a")
            for hdb in range(4):
                nc.tensor.matmul(
                    pca[:, :], oT[:, hdb, qb * 128:qb * 128 + 128],
                    wc_sb[:, hdb, 0:512], start=(hdb == 0), stop=(hdb == 3))
                yield
            stt = 4 * ss + qb
            ost = ost_p.tile([128, E], F32, tag="ost", name=f"ost{ss}q{qb}")
            nc.vector.tensor_copy(ost[:, 0:512], pca[:, :])
            nc.sync.dma_start(out=out[stt * 128:(stt + 1) * 128, 0:512],
                              in_=ost[:, 0:512])
            pcb = pool.tile([128, 512], F32, tag=ptag, name=f"pc{ss}q{qb}b")
            for hdb in range(4):
                nc.tensor.matmul(
                    pcb[:, :], oT[:, hdb, qb * 128:qb * 128 + 128],
                    wc_sb[:, hdb, 512:1024], start=(hdb == 0), stop=(hdb == 3))
                if hdb != 3:
                    yield
            nc.vector.tensor_copy(ost[:, 512:1024], pcb[:, :])
            nc.sync.dma_start(out=out[stt * 128:(stt + 1) * 128, 512:1024],
                              in_=ost[:, 512:1024])
            yield

        def attn_head(ss, h):
            m, half = h // 2, h % 2
            qt = qkt[m][64 * half:64 * half + 64, :]
            kt = qkt[4 + m][64 * half:64 * half + 64, :]
            po = po_p.tile([65, 512], F32, tag="po", name=f"po{ss}h{h}")
            blk0 = 512 * ss

            # units: fp8-DoubleRow pairs of full 512-wide k-tiles
            # (kc < 4*ss), then the 4 diagonal k-tiles in bf16, greedy-packed
            # into 1024-wide stp tiles (no chunk crosses a PSUM bank)
            chunks = []
            for kc in range(4 * ss, 4 * ss + 4):
                qlo = 128 * kc
                chunks.append((kc, qlo, 512 * (ss + 1) - qlo))
            groups, cur, off = [], [], 0
            for kc, qlo, wd in chunks:
                if cur and (off + wd > 1024 or off // 512 != (off + wd - 1) // 512):
                    groups.append(cur)
                    cur, off = [], 0
                cur.append((kc, qlo, wd, off))
                off += wd
            groups.append(cur)

            state = {"first_pv": True}

            def emit_scores_pair(j):
                stp = stp_p.tile([128, 1024], F32, tag="stp", name=f"stp{ss}h{h}")
                pt8 = pt_p.tile([128, 2, 512], FP8, tag="pt8", name=f"p8{ss}h{h}")
                for par in range(2):
                    kc = 2 * j + par
                    nc.tensor.matmul(
                        stp[:, 512 * par:512 * par + 512],
                        kt[:, 128 * kc:128 * kc + 128],
                        qt[:, blk0:blk0 + 512],
                        start=True, stop=True)
                nc.scalar.activation(
                    pt8[:, :, :].rearrange("p a b -> p (a b)"),
                    stp[:, 0:1024], ACT_EXP, bias=eshift[:, :])
                return pt8

            def emit_pv_pair(j, pt8):
                nc.tensor.matmul(
                    po[:, 0:512], v8[:, j, h, :, 0:D + 1], pt8[:, :, :],
                    start=state["first_pv"], stop=False,
                    perf_mode=mybir.MatmulPerfMode.DoubleRow,
                    skip_group_check=True)
                state["first_pv"] = False

            def emit_scores(g):
                stp = stp_p.tile([128, 1024], F32, tag="stp", name=f"stp{ss}h{h}")
                ptile = pt_p.tile([128, 1024], BF16, tag="pt", name=f"pt{ss}h{h}")
                for kc, qlo, wd, off in g:
                    nc.tensor.matmul(
                        stp[:, off:off + wd],
                        kt[:, 128 * kc:128 * kc + 128],
                        qt[:, qlo:qlo + wd],
                        start=True, stop=True)
                tot = g[-1][3] + g[-1][2]
                nc.scalar.activation(ptile[:, 0:tot], stp[:, 0:tot], ACT_EXP,
                                     bias=eshift[:, :])
                # diagonal-block causal masks (keep q >= k), off the PE path
                dgs = {}
                for kc, qlo, wd, off in g:
                    dg = dg_p.tile([128, 128], BF16, tag="dg",
                                   name=f"dg{ss}h{h}")
                    nc.gpsimd.affine_select(
                        out=dg[:, :], in_=ptile[:, off:off + 128],
                        compare_op=mybir.AluOpType.is_ge,
                        fill=0.0, base=0, pattern=[[1, 128]],
                        channel_multiplier=-1)
                    dgs[kc] = dg
                return ptile, dgs

            def pv_mm(kc, rhs, col0, ncol):
                nc.tensor.matmul(
                    po[:, col0:col0 + ncol], v_aug[:, kc, h, :], rhs,
                    start=state["first_pv"], stop=False,
                    skip_group_check=True)
                state["first_pv"] = False

            def emit_pv(g, ptile, dgs):
                # off-diagonal parts first (they only wait on exp), masked
                # diagonal blocks last (they also wait on the gpsimd mask)
                for kc, qlo, wd, off in g:
                    if wd > 128:
                        pv_mm(kc, ptile[:, off + 128:off + wd],
                              qlo + 128 - blk0, wd - 128)
                for kc, qlo, wd, off in g:
                    pv_mm(kc, dgs[kc][:, :], qlo - blk0, 128)

            def normalize():
                # normalize: oT[hd, q] = po[d, q] * (1 / po[64, q])
                rcp = rc_p.tile([1, 512], F32, tag="rc", name=f"rcp{ss}h{h}")
                nc.vector.reciprocal(rcp, po[64:65, :])
                bcst = bc_p.tile([64, 512], F32, tag="bc", name=f"bc{ss}h{h}")
                nc.gpsimd.partition_broadcast(bcst[:, :], rcp[:, :])
                if ss == 3 and h == G - 1:
                    for qb in range(4):
                        nc.vector.tensor_mul(
                            oTs[ss][64 * half:64 * half + 64, m,
                                    qb * 128:(qb + 1) * 128],
                            po[0:64, qb * 128:(qb + 1) * 128],
                            bcst[:, qb * 128:(qb + 1) * 128])
                else:
                    nc.vector.tensor_mul(
                        oTs[ss][64 * half:64 * half + 64, m, :],
                        po[0:64, :], bcst[:, :])

            units = []
            for j in range(2 * ss):
                units.append((
                    lambda j=j: ("pair", emit_scores_pair(j), j),
                    lambda sc: emit_pv_pair(sc[2], sc[1]),
                ))
            for g in groups:
                units.append((
                    lambda g=g: ("grp", emit_scores(g), g),
                    lambda sc: emit_pv(sc[2], *sc[1]),
                ))
            return units, normalize

        # ---- projections for seq block 0: interleave three QK waves
        # (6 psums borrowed across the idle attention pools) so the PE
        # consumes each contraction chunk's 6 matmuls while the next chunk's
        # DMA is in flight
        p10 = []
        for i, (pool, tg) in enumerate([(pp, "pp"), (pp, "pp"),
                                        (stp_p, "stp"), (stp_p, "stp"),
                                        (po_p, "po"), (po_p, "po")]):
            t10 = pool.tile([128, 512], F32, tag=tg, name=f"p10_{i}")
            p10.append(t10)
        ms10 = [0, 4, 1, 5, 2, 6]
        for ec in range(EC):
            for i in range(6):
                qk_mm(p10[i], ms10[i], xts[0], ec)
        for i in (0, 1, 2):
            nc.vector.tensor_copy(qkt[ms10[i]][:, 0:512], p10[i][:, :])
        for i in (3, 4, 5):
            nc.scalar.activation(qkt[ms10[i]][:, 0:512], p10[i][:, :],
                                 mybir.ActivationFunctionType.Copy)
        for _ in ph1_qk_wave(0, 3):
            pass
        for w in range(2):
            for _ in ph1_v_wave(0, w):
                pass

        # ---- main sweep over 512-seq blocks
        for ss in range(4):
            if ss < 3:
                xts[ss + 1] = xt_p.tile([128, EC, 512], BF16, tag="xt",
                                        name=f"xt{ss + 1}")
                for ec in range(EC):
                    nc.sync.dma_start(
                        out=xts[ss + 1][:, ec, :],
                        in_=xT[ec][:, (ss + 1) * 512:(ss + 2) * 512])
            oTs[ss] = oT_p.tile([128, 4, 512], BF16, tag="oT", name=f"oT{ss}")

            # PE filler work pulled between attention groups: blocks 0-2
            # get the next block's projections, the last block gets all the
            # deferred c_proj tiles (it has no projections left to run)
            gen_list = []
            if ss < 3:
                for w in range(4):
                    gen_list.append(ph1_qk_wave(ss + 1, w))
                for w in range(2):
                    gen_list.append(ph1_v_wave(ss + 1, w))
                n_steps_total = 48
            else:
                for pss in range(3):
                    for qb in range(4):
                        gen_list.append(ph3_tile(pss, qb))
                n_steps_total = 96

            gen_iter = iter(gen_list)
            current = {"g": None}

            def pull_one():
                while True:
                    if current["g"] is None:
                        current["g"] = next(gen_iter, None)
                        if current["g"] is None:
                            return False
                    try:
                        next(current["g"])
                        return True
                    except StopIteration:
                        current["g"] = None

            # units per head: 2*ss fp8 pairs + 2 bf16 diagonal groups
            _g = 2 * ss + 2
            head_lo = 1 if ss in (1, 2) else 0
            total_groups = (8 - head_lo) * _g

            pull_count = {"n": 0, "done": 0}

            def make_pull(active):
                def pull():
                    if not active:
                        return
                    pull_count["n"] += 1
                    target = (n_steps_total * pull_count["n"] + total_groups - 1) \
                        // max(total_groups, 1)
                    while pull_count["done"] < target:
                        if not pull_one():
                            return
                        pull_count["done"] += 1
                return pull

            # global software pipeline: scores of unit i+1 are emitted
            # before PV of unit i, across head boundaries, so the PE always
            # has queued work while ACT computes the exp
            pend = None
            for h in range(G):
                units, normalize = attn_head(ss, h)
                pull = make_pull(h >= head_lo)
                for i, (emit_sc, emit_pv_u) in enumerate(units):
                    sc = emit_sc()
                    if pend is not None:
                        pend[0](pend[1])
                        if pend[2] is not None:
                            pend[2]()
                        pend[3]()
                    pend = (emit_pv_u, sc,
                            normalize if i == len(units) - 1 else None, pull)
            if pend is not None:
                pend[0](pend[1])
                if pend[2] is not None:
                    pend[2]()
                pend[3]()
            # drain any remaining filler steps
            while pull_one():
                pass

        # final block's c_proj
        for qb in range(4):
            for _ in ph3_tile(3, qb):
                pass

    nc.compile()
    return nc


def prep_core_inputs(hidden_states, position_states, Wq, bq, Wqh, bqh, Wk, bk,
                     Wkh, bkh, Wv, bv, Wvh, bvh, Wp, bp, Wpe, bpe, Wc, bc):
    """Build the per-core input maps (host-side weight folding + sharding)."""
    bf16 = ml_dtypes.bfloat16
    f32 = np.float32

    def fused(parity):
        hs = slice(G * parity, G * parity + G)
        mats = {}
        for name, (Wa, ba, Wh, bh, v) in {
            "q": (Wq, bq, Wqh[hs], bqh[hs], 0),
            "k": (Wk, bk, Wkh[hs], bkh[hs], 1),
            "v": (Wv, bv, Wvh[hs], bvh[hs], 2),
        }.items():
            mx = np.einsum("hed,ghd->hegd", Wa, Wh).reshape(E, QKD)
            mp = np.einsum("pd,g->pgd", Wp[:, v * D:(v + 1) * D], Wpe[v, 0, hs]).reshape(P, QKD)
            bias = (np.einsum("hd,ghd->gd", ba, Wh) + bh
                    + bp[v * D:(v + 1) * D][None, :] * Wpe[v, 0, hs][:, None]
                    + bpe[hs][:, None]).reshape(QKD)
            if name == "q":
                sc = 1.0 / np.sqrt(np.float32(D))
                mx, mp, bias = mx * sc, mp * sc, bias * sc
            mats[name] = (mx, mp, bias)

        def chunks(mx, mp, bias):
            w = mx.shape[1]
            m9 = np.zeros((EC, 128, w), f32)
            m9[:8] = mx.reshape(8, 128, w)
            m9[8, :P] = mp
            m9[8, P] = bias
            return m9
        mqk9 = np.concatenate([chunks(*mats["q"]), chunks(*mats["k"])], axis=2)
        mv9 = chunks(*mats["v"])
        wc4 = Wc.reshape(H, D, E)[hs].reshape(QKD, E).reshape(4, 128, E)
        return (np.ascontiguousarray(mqk9).astype(bf16),
                np.ascontiguousarray(mv9).astype(bf16),
                np.ascontiguousarray(wc4).astype(bf16))

    per_parity = [fused(0), fused(1)]

    in_maps = []
    for c in range(NC):
        b, parity = c // 2, c % 2
        x9 = np.zeros((EC, 128, S), f32)
        x9[:8] = np.ascontiguousarray(hidden_states[b].T).reshape(8, 128, S)
        x9[8, :P] = position_states[b].T
        x9[8, P] = 1.0
        mqk9, mv9, wc4 = per_parity[parity]
        in_maps.append({"xT": x9.astype(bf16), "Mqk": mqk9, "Mv": mv9,
                        "Wc": wc4})
    return in_maps


_NC_CACHE = {}


def get_nc():
    if "nc" not in _NC_CACHE:
        _NC_CACHE["nc"] = build_nc()
    return _NC_CACHE["nc"]


def kernel(**inputs):
    nc = get_nc()
    in_maps = prep_core_inputs(**inputs)
    res = run_bass_kernel_spmd(nc, in_maps, list(range(NC)))
    bc = inputs["bc"]
    outs = [res.results[2 * b]["out"] + res.results[2 * b + 1]["out"] + bc
            for b in range(B)]
    return np.stack(outs).astype(np.float32)
